# revision 4
# baseline (speedup 1.0000x reference)
"""ACDC layer on 8 TRN2 NeuronCores — exact-S + fp8-F split, all on the PE.

Math: per group g (4 groups of 1024 cols), out = x @ M_g^T + bias with
M_g = Cinv @ P_riffle @ diag(D) @ C @ diag(A). Rows of M_g concentrate on two
lines: L1 col=floor(r/2) and its mirror L2 col=1023-floor(r/2). Split
M = S + F:
  - S: two 64-wide column blocks per 128-row band (the L1/L2 neighborhoods,
    ~99% of the Frobenius energy) -> one exact f16 matmul per output chunk.
  - F: the remainder (11% of the norm) -> dense fp8(e4m3) DoubleRow matmuls
    at ~1.5-1.8x the bf16 rate. fp8 noise only touches F: total rel err
    ~4.4e-3 (validated on host against f64).

Output basis is permuted so each 128-lane output chunk (even rows | odd rows
of one 64-block) draws its whole S-support from ONE 128-lane source chunk
(lo cols | mirrored hi cols), making S a single [128,128] stationary per
chunk that accumulates into the same PSUM as the fp8 matmuls. x is uploaded
pre-transposed exactly once, in f16 (no PE transposes anywhere); the e4m3
copy for the F-matmuls is derived on device by a DVE cast (x32 scale), with
the permuted lane order baked into W on the host. Power-of-2 scale
compensation and the bias are per-partition scalars folded into the
single-op PSUM drain (DVE/ACT alternating); the host un-permutes and
transposes the f16 output. Loads stream as chunk-pairs across both HWDGE
rings with a 7-pair lookahead; stores go out as unit-pairs (8 KiB runs).
"""

import numpy as np
import ml_dtypes

import concourse.bass as bass
import concourse.tile as tile
from concourse import bacc, mybir
from concourse.bass_utils import run_bass_kernel_spmd

N_CORES = 8
N_FULL, D = 16384, 4096
GROUPS = 4
DG = 1024
ROWS = N_FULL // N_CORES  # 2048
P = 128
RB = 4                    # row blocks per core
RBS = ROWS // RB          # 512
SX = 32.0                 # fp8 scale for x

_CACHE: dict = {}
LAST_RESULT = None


def _ensure_profile_hook_module():
    try:
        import antenv.axon_hooks  # noqa: F401
    except Exception:
        try:
            import sys
            import types

            import antenv

            m = types.ModuleType("antenv.axon_hooks")
            m._h = None
            m.get_axon_ntff_profile_hook = lambda: m._h
            m.set_axon_ntff_profile_hook = lambda h: setattr(m, "_h", h)
            sys.modules["antenv.axon_hooks"] = m
            antenv.axon_hooks = m
        except Exception:
            pass


def _out_perm():
    """Per group: device out-row q = 128*ch + lam  ->  group col r.
    lam<64: r = 2*(64*ch+lam) (even rows); lam>=64: r = 2*(64*ch+lam-64)+1."""
    q = np.arange(DG)
    ch = q // P
    lam = q % P
    return 2 * (64 * ch + (lam % 64)) + (lam >= 64)


def _src_cols():
    """Per group: x16 source chunk cc, lane s -> group col.
    s<64: col = 64*cc+s (lo); s>=64: col = 1023-(64*cc+s-64) (mirrored hi)."""
    cols = np.empty((8, P), np.int64)
    for cc in range(8):
        s = np.arange(64)
        cols[cc, :64] = 64 * cc + s
        cols[cc, 64:] = 1023 - (64 * cc + s)
    return cols


def _prep_weights(A, Dv, bias):
    """Build W8 [128,32,1024] e4m3, T [128,32,128] f16, coef [128,32,2] f32.

    coef slots: 0 = bias per out-lane, 1 = 1/(sW*SX) drain scale.
    """
    N = DG
    j = np.arange(N)[None, :]
    k = np.arange(N)[:, None]
    C = 2.0 * np.cos(np.pi * (j + 0.5) * k / N)
    kk = np.arange(N)[None, :]
    jj = np.arange(N)[:, None]
    w0 = np.ones(N)
    w0[0] = 0.5
    Cinv = (1.0 / N) * w0[None, :] * np.cos(np.pi * kk * (jj + 0.5) / N)
    perm = np.arange(N).reshape(N // 2, 2).T.reshape(N)

    rperm = _out_perm()           # device out-row -> group col
    scols = _src_cols()           # [8, 128] source chunk lane -> group col
    mask = np.zeros((N, N), bool)
    for ch in range(8):
        rows = slice(P * ch, P * ch + P)
        mask[rows, 64 * ch:64 * ch + 64] = True
        mask[rows, N - 64 * ch - 64:N - 64 * ch] = True

    W8 = np.empty((P, 32, N), dtype=ml_dtypes.float8_e4m3)
    T = np.empty((P, 32, P), dtype=np.float16)
    coef = np.zeros((P, 32, 2), dtype=np.float32)
    for g in range(GROUPS):
        Ag = A[0, g * N:(g + 1) * N].astype(np.float64)
        Dg = Dv[0, g * N:(g + 1) * N].astype(np.float64)
        M = Cinv @ ((Dg[:, None] * C * Ag[None, :])[perm])
        S = np.where(mask, M, 0.0)
        F = M - S
        sW = 2.0 ** np.floor(np.log2(224.0 / np.abs(F).max()))
        coef[:, g * 8:(g + 1) * 8, 1] = 1.0 / (sW * SX)
        # F^T with K in the permuted x16 lane order (x8 is derived from
        # x16 on device): W8[p, g*8+cc, n] = sW*F[rperm[n], scols[cc, p]]
        Fr = F[rperm, :] * sW                      # [n, col]
        for cc in range(8):
            W8[:, g * 8 + cc, :] = np.clip(
                Fr[:, scols[cc]].T, -240, 240
            ).astype(ml_dtypes.float8_e4m3)
        for ch in range(8):
            o = g * 8 + ch
            rows = rperm[P * ch:P * ch + P]          # out-lane -> group col
            # T[sigma, o, lam] = sW*SX * M[rows[lam], scols[ch, sigma]]
            T[:, o, :] = (
                (M[np.ix_(rows, scols[ch])] * (sW * SX)).T.astype(np.float16)
            )
            coef[:, o, 0] = bias[0, g * N + rows]
    return W8, T, coef


def _prep_x(xc):
    """xc [2048, 4096] f32 -> x16 [4,128,8,2048] f16 (64-block lo|hi-rev).
    The fp8 copy for the F-matmuls is derived on device (DVE cast)."""
    xg = np.ascontiguousarray(xc.T).reshape(GROUPS, DG, ROWS)    # [g, col, r]
    scols = _src_cols()
    x16 = xg[:, scols, :].transpose(0, 2, 1, 3).astype(np.float16)  # [g,p,cc,r]
    return np.ascontiguousarray(x16)


_COLMAP = None


def _post(out_dev):
    """out_dev [32, 128, 2048] f16 -> [2048, 4096] f32, original col order."""
    global _COLMAP
    if _COLMAP is None:
        rp = _out_perm()
        _COLMAP = (np.arange(GROUPS)[:, None] * DG + rp[None, :]).reshape(-1)
    out = np.empty((ROWS, D), np.float32)
    dev = out_dev.reshape(16, P, 2, ROWS).transpose(0, 2, 1, 3).reshape(D, ROWS)
    out[:, _COLMAP] = dev.T.astype(np.float32)
    return out


def _build_kernel():
    nc = bacc.Bacc("TRN2", target_bir_lowering=False, debug=False)

    x16_ext = nc.declare_dram_parameter(
        "x16", [GROUPS, P, 8, ROWS], mybir.dt.float16, isOutput=False
    )
    w_ext = nc.declare_dram_parameter(
        "w", [P, 32, DG], mybir.dt.float8e4, isOutput=False
    )
    t_ext = nc.declare_dram_parameter(
        "t", [P, 32, P], mybir.dt.float16, isOutput=False
    )
    coef_ext = nc.declare_dram_parameter(
        "coef", [P, 32, 2], mybir.dt.float32, isOutput=False
    )
    out_ext = nc.declare_dram_parameter(
        "out", [16, P, 2, ROWS], mybir.dt.float16, isOutput=True
    )

    DR = mybir.MatmulPerfMode.DoubleRow
    MUL = mybir.AluOpType.mult
    ADD = mybir.AluOpType.add

    with tile.TileContext(nc) as tc:
        with (
            tc.tile_pool(name="consts", bufs=1) as consts,
            tc.tile_pool(name="x16p", bufs=9) as x16p,
            tc.tile_pool(name="op", bufs=3) as op,
            tc.tile_pool(name="pa", bufs=8, space=bass.MemorySpace.PSUM) as pa,
        ):
            coef_sb = consts.tile([P, 32, 2], mybir.dt.float32)
            nc.sync.dma_start(out=coef_sb, in_=coef_ext[:])
            w_sb = consts.tile([P, 32, DG], mybir.dt.float8e4)
            x8_sb = consts.tile([P, 32, ROWS], mybir.dt.float8e4)
            t_sb = consts.tile([P, 32, P], mybir.dt.float16)

            nc.sync.dma_start(out=w_sb[:, 0:8, :], in_=w_ext[:, 0:8, :])

            # software-pipelined loads: sync-ring emission order == need
            # order ([W-g, x8-g] then that group's x16 chunks), issued
            # LOOKAHEAD pairs ahead of use so the ring never HOL-blocks
            NPAIR = GROUPS * 4
            LOOKAHEAD = 7
            x16c: dict = {}

            def emit_loads(pi):
                g, cA = pi // 4, 2 * (pi % 4)
                if cA == 0 and g > 0:
                    nc.sync.dma_start(
                        out=w_sb[:, g * 8:(g + 1) * 8, :],
                        in_=w_ext[:, g * 8:(g + 1) * 8, :],
                    )
                xc = x16p.tile([P, 2, ROWS], mybir.dt.float16, name="x16c")
                ring = nc.scalar if (pi & 1) == 0 else nc.sync
                ring.dma_start(out=xc, in_=x16_ext[g][:, cA:cA + 2, :])
                x16c[(g, cA)] = xc
                # derive the fp8 copy for the F-matmuls on device
                nc.vector.tensor_scalar_mul(
                    out=x8_sb[:, g * 8 + cA:g * 8 + cA + 2, :],
                    in0=xc, scalar1=SX,
                )

            for pi in range(min(LOOKAHEAD, NPAIR)):
                emit_loads(pi)
                if pi == 0:
                    nc.scalar.dma_start(out=t_sb, in_=t_ext[:])

            # process units in pairs to halve fp8<->f16 perf-mode switches;
            # a pair holds all 8 PSUM banks, drains release them mid-pair
            for pidx in range(NPAIR):
                if pidx + LOOKAHEAD < NPAIR:
                    emit_loads(pidx + LOOKAHEAD)
                g, chA = pidx // 4, 2 * (pidx % 4)
                pair = (chA, chA + 1)
                psums = {}
                for ch in pair:
                    for rb in range(RB):
                        psums[(ch, rb)] = pa.tile(
                            [P, RBS], mybir.dt.float32, name="ps"
                        )
                for ch in pair:
                    for kp in range(4):
                        wsl = w_sb[:, g * 8 + 2 * kp:g * 8 + 2 * kp + 2,
                                   ch * P:(ch + 1) * P]
                        for rb in range(RB):
                            nc.tensor.matmul(
                                psums[(ch, rb)],
                                lhsT=wsl,
                                rhs=x8_sb[:, g * 8 + 2 * kp:
                                          g * 8 + 2 * kp + 2,
                                          rb * RBS:(rb + 1) * RBS],
                                start=(kp == 0),
                                stop=False,
                                perf_mode=DR,
                            )
                # exact-S f16 matmuls into the same PSUMs (T pre-scaled
                # by sW*SX so one power-of-2 descale covers both paths)
                for ch in pair:
                    for rb in range(RB):
                        nc.tensor.matmul(
                            psums[(ch, rb)],
                            lhsT=t_sb[:, g * 8 + ch, :],
                            rhs=x16c[(g, chA)][:, ch - chA,
                                               rb * RBS:(rb + 1) * RBS],
                            start=False,
                            stop=True,
                        )
                ost = op.tile([P, 2, ROWS], mybir.dt.float16, name="ost")
                last = pidx == NPAIR - 1
                for ch in pair:
                    o = g * 8 + ch
                    u = ch - chA
                    for rb in range(RB):
                        # drain: out = psum*1/(sW*SX) + bias (one op);
                        # alternate DVE/ACT so drains run in parallel
                        if (ch + (rb if last else 0)) & 1 == 0:
                            nc.vector.tensor_scalar(
                                out=ost[:, u, rb * RBS:(rb + 1) * RBS],
                                in0=psums[(ch, rb)],
                                scalar1=coef_sb[:, o, 1:2],
                                scalar2=coef_sb[:, o, 0:1],
                                op0=MUL, op1=ADD,
                            )
                        else:
                            nc.scalar.activation(
                                out=ost[:, u, rb * RBS:(rb + 1) * RBS],
                                in_=psums[(ch, rb)],
                                func=mybir.ActivationFunctionType.Identity,
                                bias=coef_sb[:, o, 0:1],
                                scale=coef_sb[:, o, 1:2],
                            )
                        if last:
                            # final pair drains+stores per-rb so the kernel
                            # tail only waits on a 128KiB transfer
                            nc.scalar.dma_start(
                                out=out_ext[pidx, :, u,
                                            rb * RBS:(rb + 1) * RBS],
                                in_=ost[:, u, rb * RBS:(rb + 1) * RBS],
                            )
                if not last:
                    # one pair store: 8KiB contiguous per lane
                    nc.scalar.dma_start(out=out_ext[pidx], in_=ost)

    nc.compile()
    return nc


def kernel(x, A, D, bias):
    global LAST_RESULT
    x = np.ascontiguousarray(np.asarray(x, dtype=np.float32))
    W8, T, coef = _prep_weights(
        np.asarray(A, np.float32), np.asarray(D, np.float32),
        np.asarray(bias, np.float32),
    )

    _ensure_profile_hook_module()
    if "nc" not in _CACHE:
        _CACHE["nc"] = _build_kernel()
    nc = _CACHE["nc"]

    in_maps = []
    for i in range(N_CORES):
        x16 = _prep_x(x[i * ROWS:(i + 1) * ROWS])
        in_maps.append({"x16": x16, "w": W8, "t": T, "coef": coef})
    res = run_bass_kernel_spmd(nc, in_maps, core_ids=list(range(N_CORES)))
    LAST_RESULT = res
    out = np.concatenate(
        [_post(res.results[i]["out"]) for i in range(N_CORES)], axis=0
    )
    return out


# revision 5
# speedup vs baseline: 1.0259x; 1.0259x over previous
"""ACDC layer on 8 TRN2 NeuronCores — exact-S + fp8-F split, all on the PE.

Math: per group g (4 groups of 1024 cols), out = x @ M_g^T + bias with
M_g = Cinv @ P_riffle @ diag(D) @ C @ diag(A). Rows of M_g concentrate on two
lines: L1 col=floor(r/2) and its mirror L2 col=1023-floor(r/2). Split
M = S + F:
  - S: two 64-wide column blocks per 128-row band (the L1/L2 neighborhoods,
    ~99% of the Frobenius energy) -> one exact f16 matmul per output chunk.
  - F: the remainder (11% of the norm) -> dense fp8(e4m3) DoubleRow matmuls
    at ~1.5-1.8x the bf16 rate. fp8 noise only touches F: total rel err
    ~4.4e-3 (validated on host against f64).

Output basis is permuted so each 128-lane output chunk (even rows | odd rows
of one 64-block) draws its whole S-support from ONE 128-lane source chunk
(lo cols | mirrored hi cols), making S a single [128,128] stationary per
chunk that accumulates into the same PSUM as the fp8 matmuls. x is uploaded
pre-transposed exactly once, in f16 (no PE transposes anywhere); the e4m3
copy for the F-matmuls is derived on device by a DVE cast (x32 scale), with
the permuted lane order baked into W on the host. Power-of-2 scale
compensation and the bias are per-partition scalars folded into the
single-op PSUM drain (DVE/ACT alternating); the host un-permutes and
transposes the f16 output. Loads stream as chunk-pairs across both HWDGE
rings with a 7-pair lookahead; stores go out as unit-pairs (8 KiB runs).
"""

import numpy as np
import ml_dtypes

import concourse.bass as bass
import concourse.tile as tile
from concourse import bacc, mybir
from concourse.bass_utils import run_bass_kernel_spmd

N_CORES = 8
N_FULL, D = 16384, 4096
GROUPS = 4
DG = 1024
ROWS = N_FULL // N_CORES  # 2048
P = 128
RB = 4                    # row blocks per core
RBS = ROWS // RB          # 512
SX = 32.0                 # fp8 scale for x

_CACHE: dict = {}
LAST_RESULT = None


def _ensure_profile_hook_module():
    try:
        import antenv.axon_hooks  # noqa: F401
    except Exception:
        try:
            import sys
            import types

            import antenv

            m = types.ModuleType("antenv.axon_hooks")
            m._h = None
            m.get_axon_ntff_profile_hook = lambda: m._h
            m.set_axon_ntff_profile_hook = lambda h: setattr(m, "_h", h)
            sys.modules["antenv.axon_hooks"] = m
            antenv.axon_hooks = m
        except Exception:
            pass


def _out_perm():
    """Per group: device out-row q = 128*ch + lam  ->  group col r.
    lam<64: r = 2*(64*ch+lam) (even rows); lam>=64: r = 2*(64*ch+lam-64)+1."""
    q = np.arange(DG)
    ch = q // P
    lam = q % P
    return 2 * (64 * ch + (lam % 64)) + (lam >= 64)


def _src_cols():
    """Per group: x16 source chunk cc, lane s -> group col.
    s<64: col = 64*cc+s (lo); s>=64: col = 1023-(64*cc+s-64) (mirrored hi)."""
    cols = np.empty((8, P), np.int64)
    for cc in range(8):
        s = np.arange(64)
        cols[cc, :64] = 64 * cc + s
        cols[cc, 64:] = 1023 - (64 * cc + s)
    return cols


def _prep_weights(A, Dv, bias):
    """Build W8 [128,32,1024] e4m3, T [128,32,128] f16, coef [128,32,2] f32.

    coef slots: 0 = bias per out-lane, 1 = 1/(sW*SX) drain scale.
    """
    N = DG
    j = np.arange(N)[None, :]
    k = np.arange(N)[:, None]
    C = 2.0 * np.cos(np.pi * (j + 0.5) * k / N)
    kk = np.arange(N)[None, :]
    jj = np.arange(N)[:, None]
    w0 = np.ones(N)
    w0[0] = 0.5
    Cinv = (1.0 / N) * w0[None, :] * np.cos(np.pi * kk * (jj + 0.5) / N)
    perm = np.arange(N).reshape(N // 2, 2).T.reshape(N)

    rperm = _out_perm()           # device out-row -> group col
    scols = _src_cols()           # [8, 128] source chunk lane -> group col
    mask = np.zeros((N, N), bool)
    for ch in range(8):
        rows = slice(P * ch, P * ch + P)
        mask[rows, 64 * ch:64 * ch + 64] = True
        mask[rows, N - 64 * ch - 64:N - 64 * ch] = True

    W8 = np.empty((P, 32, N), dtype=ml_dtypes.float8_e4m3)
    T = np.empty((P, 32, P), dtype=np.float16)
    coef = np.zeros((P, 32, 2), dtype=np.float32)
    for g in range(GROUPS):
        Ag = A[0, g * N:(g + 1) * N].astype(np.float64)
        Dg = Dv[0, g * N:(g + 1) * N].astype(np.float64)
        M = Cinv @ ((Dg[:, None] * C * Ag[None, :])[perm])
        S = np.where(mask, M, 0.0)
        F = M - S
        sW = 2.0 ** np.floor(np.log2(224.0 / np.abs(F).max()))
        coef[:, g * 8:(g + 1) * 8, 1] = 1.0 / (sW * SX)
        # F^T with K in the permuted x16 lane order (x8 is derived from
        # x16 on device): W8[p, g*8+cc, n] = sW*F[rperm[n], scols[cc, p]]
        Fr = F[rperm, :] * sW                      # [n, col]
        for cc in range(8):
            W8[:, g * 8 + cc, :] = np.clip(
                Fr[:, scols[cc]].T, -240, 240
            ).astype(ml_dtypes.float8_e4m3)
        for ch in range(8):
            o = g * 8 + ch
            rows = rperm[P * ch:P * ch + P]          # out-lane -> group col
            # T[sigma, o, lam] = sW*SX * M[rows[lam], scols[ch, sigma]]
            T[:, o, :] = (
                (M[np.ix_(rows, scols[ch])] * (sW * SX)).T.astype(np.float16)
            )
            coef[:, o, 0] = bias[0, g * N + rows]
    return W8, T, coef


def _prep_x(xc):
    """xc [2048, 4096] f32 -> x16 [4,128,8,2048] f16 (64-block lo|hi-rev).
    The fp8 copy for the F-matmuls is derived on device (DVE cast)."""
    xg = np.ascontiguousarray(xc.T).reshape(GROUPS, DG, ROWS)    # [g, col, r]
    scols = _src_cols()
    x16 = xg[:, scols, :].transpose(0, 2, 1, 3).astype(np.float16)  # [g,p,cc,r]
    return np.ascontiguousarray(x16)


_COLMAP = None


def _post(out_dev):
    """out_dev [32, 128, 2048] f16 -> [2048, 4096] f32, original col order."""
    global _COLMAP
    if _COLMAP is None:
        rp = _out_perm()
        _COLMAP = (np.arange(GROUPS)[:, None] * DG + rp[None, :]).reshape(-1)
    out = np.empty((ROWS, D), np.float32)
    dev = out_dev.reshape(16, P, 2, ROWS).transpose(0, 2, 1, 3).reshape(D, ROWS)
    out[:, _COLMAP] = dev.T.astype(np.float32)
    return out


def _build_kernel():
    nc = bacc.Bacc("TRN2", target_bir_lowering=False, debug=False)

    x16_ext = nc.declare_dram_parameter(
        "x16", [GROUPS, P, 8, ROWS], mybir.dt.float16, isOutput=False
    )
    w_ext = nc.declare_dram_parameter(
        "w", [P, 32, DG], mybir.dt.float8e4, isOutput=False
    )
    t_ext = nc.declare_dram_parameter(
        "t", [P, 32, P], mybir.dt.float16, isOutput=False
    )
    coef_ext = nc.declare_dram_parameter(
        "coef", [P, 32, 2], mybir.dt.float32, isOutput=False
    )
    out_ext = nc.declare_dram_parameter(
        "out", [16, P, 2, ROWS], mybir.dt.float16, isOutput=True
    )

    DR = mybir.MatmulPerfMode.DoubleRow
    MUL = mybir.AluOpType.mult
    ADD = mybir.AluOpType.add

    with tile.TileContext(nc) as tc:
        with (
            tc.tile_pool(name="consts", bufs=1) as consts,
            tc.tile_pool(name="x16p", bufs=9) as x16p,
            tc.tile_pool(name="op", bufs=3) as op,
            tc.tile_pool(name="pa", bufs=8, space=bass.MemorySpace.PSUM) as pa,
        ):
            coef_sb = consts.tile([P, 32, 2], mybir.dt.float32)
            nc.sync.dma_start(out=coef_sb, in_=coef_ext[:])
            w_sb = consts.tile([P, 32, DG], mybir.dt.float8e4)
            x8_sb = consts.tile([P, 32, ROWS], mybir.dt.float8e4)
            t_sb = consts.tile([P, 32, P], mybir.dt.float16)

            nc.sync.dma_start(out=w_sb[:, 0:8, :], in_=w_ext[:, 0:8, :])

            # software-pipelined loads: sync-ring emission order == need
            # order ([W-g, x8-g] then that group's x16 chunks), issued
            # LOOKAHEAD pairs ahead of use so the ring never HOL-blocks
            NPAIR = GROUPS * 4
            LOOKAHEAD = 7
            x16c: dict = {}

            def emit_loads(pi):
                g, cA = pi // 4, 2 * (pi % 4)
                if cA == 0 and g > 0:
                    nc.sync.dma_start(
                        out=w_sb[:, g * 8:(g + 1) * 8, :],
                        in_=w_ext[:, g * 8:(g + 1) * 8, :],
                    )
                xc = x16p.tile([P, 2, ROWS], mybir.dt.float16, name="x16c")
                ring = nc.scalar if (pi & 1) == 0 else nc.sync
                ring.dma_start(out=xc, in_=x16_ext[g][:, cA:cA + 2, :])
                x16c[(g, cA)] = xc
                # derive the fp8 copy for the F-matmuls on device
                nc.vector.tensor_scalar_mul(
                    out=x8_sb[:, g * 8 + cA:g * 8 + cA + 2, :],
                    in0=xc, scalar1=SX,
                )

            for pi in range(min(LOOKAHEAD, NPAIR)):
                emit_loads(pi)
                if pi == 0:
                    nc.scalar.dma_start(out=t_sb, in_=t_ext[:])

            # process units in pairs to halve fp8<->f16 perf-mode switches;
            # a pair holds all 8 PSUM banks, drains release them mid-pair
            for pidx in range(NPAIR):
                if pidx + LOOKAHEAD < NPAIR:
                    emit_loads(pidx + LOOKAHEAD)
                g, chA = pidx // 4, 2 * (pidx % 4)
                pair = (chA, chA + 1)
                psums = {}
                for ch in pair:
                    for rb in range(RB):
                        psums[(ch, rb)] = pa.tile(
                            [P, RBS], mybir.dt.float32, name="ps"
                        )
                # exact-S f16 matmuls first (they only need this pair's
                # x16 chunk, so they run while the group's remaining fp8
                # chunks stream in); T is pre-scaled by sW*SX so one
                # power-of-2 descale covers both paths
                for ch in pair:
                    for rb in range(RB):
                        nc.tensor.matmul(
                            psums[(ch, rb)],
                            lhsT=t_sb[:, g * 8 + ch, :],
                            rhs=x16c[(g, chA)][:, ch - chA,
                                               rb * RBS:(rb + 1) * RBS],
                            start=True,
                            stop=False,
                        )
                for ch in pair:
                    for kp in range(4):
                        wsl = w_sb[:, g * 8 + 2 * kp:g * 8 + 2 * kp + 2,
                                   ch * P:(ch + 1) * P]
                        for rb in range(RB):
                            nc.tensor.matmul(
                                psums[(ch, rb)],
                                lhsT=wsl,
                                rhs=x8_sb[:, g * 8 + 2 * kp:
                                          g * 8 + 2 * kp + 2,
                                          rb * RBS:(rb + 1) * RBS],
                                start=False,
                                stop=(kp == 3),
                                perf_mode=DR,
                            )
                ost = op.tile([P, 2, ROWS], mybir.dt.float16, name="ost")
                last = pidx == NPAIR - 1
                for ch in pair:
                    o = g * 8 + ch
                    u = ch - chA
                    for rb in range(RB):
                        # drain: out = psum*1/(sW*SX) + bias (one op);
                        # alternate DVE/ACT so drains run in parallel
                        if (ch + (rb if last else 0)) & 1 == 0:
                            nc.vector.tensor_scalar(
                                out=ost[:, u, rb * RBS:(rb + 1) * RBS],
                                in0=psums[(ch, rb)],
                                scalar1=coef_sb[:, o, 1:2],
                                scalar2=coef_sb[:, o, 0:1],
                                op0=MUL, op1=ADD,
                            )
                        else:
                            nc.scalar.activation(
                                out=ost[:, u, rb * RBS:(rb + 1) * RBS],
                                in_=psums[(ch, rb)],
                                func=mybir.ActivationFunctionType.Identity,
                                bias=coef_sb[:, o, 0:1],
                                scale=coef_sb[:, o, 1:2],
                            )
                        if last:
                            # final pair drains+stores per-rb so the kernel
                            # tail only waits on a 128KiB transfer
                            nc.scalar.dma_start(
                                out=out_ext[pidx, :, u,
                                            rb * RBS:(rb + 1) * RBS],
                                in_=ost[:, u, rb * RBS:(rb + 1) * RBS],
                            )
                if not last:
                    # one pair store: 8KiB contiguous per lane
                    nc.scalar.dma_start(out=out_ext[pidx], in_=ost)

    nc.compile()
    return nc


def kernel(x, A, D, bias):
    global LAST_RESULT
    x = np.ascontiguousarray(np.asarray(x, dtype=np.float32))
    W8, T, coef = _prep_weights(
        np.asarray(A, np.float32), np.asarray(D, np.float32),
        np.asarray(bias, np.float32),
    )

    _ensure_profile_hook_module()
    if "nc" not in _CACHE:
        _CACHE["nc"] = _build_kernel()
    nc = _CACHE["nc"]

    in_maps = []
    for i in range(N_CORES):
        x16 = _prep_x(x[i * ROWS:(i + 1) * ROWS])
        in_maps.append({"x16": x16, "w": W8, "t": T, "coef": coef})
    res = run_bass_kernel_spmd(nc, in_maps, core_ids=list(range(N_CORES)))
    LAST_RESULT = res
    out = np.concatenate(
        [_post(res.results[i]["out"]) for i in range(N_CORES)], axis=0
    )
    return out
